# revision 63
# baseline (speedup 1.0000x reference)
"""LogSparse attention kernel for 8 TRN2 NeuronCores.

Problem: B=4, S=2048, H=1024, 16 heads x 64 dim. Logsparse mask: query i
attends key j iff i-j == 0 or i-j == 2^k (so <=12 keys per query, at
power-of-2 offsets).

Sharding: core c -> batch b = c//2, head-group g = c%2 (8 heads each).
Each core computes q/k/v projections for its (batch, head-group) and the
sparse attention, writing out[b, :, g*512:(g+1)*512].

Device algorithm (per core):
  - DMA-transpose X (bf16, two halves on the two DMA queues) -> XT [h, s].
  - QT/KT = W @ XT ([dh, s], dh on partitions), V = X @ WvT (s-major,
    with a ones column for row sums). After each 128-row slab of QT/KT,
    an SBUF->SBUF xbar transpose produces s-major per-slab copies
    qs_t/ks_t [s%128, blk, 128] (whole-tile transposes only: sliced
    transpose outputs and DRAM round-trips have unreliable DMA ordering).
  - Far diagonals (offsets 256/512/1024) only need diag(Q Kshift^T):
    batched DVE products of s-major q/k over all query blocks + one
    segmented tensor_reduce per (slab, offset) -> scores [si, qb, d, 2],
    exp'd on ACT, relayed out to qb-major via one gpsimd copy. All of it
    runs during the remaining projections on otherwise-idle engines.
    (Reduce/exp slices stay contiguous: multi-dim strided tiny-run
    outputs mis-execute on ACT/DVE.)
  - Dense attention is key-block-major: key block kb serves query blocks
    kb and kb+1 (256 score columns). Per kb: 8 score matmuls (K=64,
    row-tiled pairs into different psum banks, start= only on the first
    matmul per bank), then the logsparse mask is ADDED on the PE itself
    (identity-stationary matmuls adding -1e9/8*am log-masks into the
    psum) so exp(0.125*(s+M)) masks exactly to 0 with no vector-engine
    work; two batched exp ACTs (psum->bf16) per kb.
  - PV: per qb, 16 matmuls (2 strips x 8 heads, N=65 incl rowsum col)
    accumulate into 2 psum banks; far p*v rank-1 MACs (broadcast APs),
    psum+far combine, reciprocal and normalize are batched per qb on DVE.
Softmax max-subtraction is skipped: scores*0.125 has std ~0.4 for this
problem family, far from exp overflow.
"""

import numpy as np
import ml_dtypes

import concourse.bass as bass
from concourse import bacc
import concourse.mybir as mybir
from concourse.tile import TileContext
from concourse.bass_utils import run_bass_kernel_spmd

B, S, H = 4, 2048, 1024
NH, HD = 16, 64
G = 2  # head groups per batch
HPC = NH // G  # heads per core = 8
GD = HPC * HD  # 512 group dim
NQB = S // 128  # 16 query blocks
KCH = H // 128  # 8 contraction chunks

BF16 = mybir.dt.bfloat16
F32 = mybir.dt.float32
NPBF16 = ml_dtypes.bfloat16

FAR = (2, 4, 8)  # far diagonal offsets in 128-blocks (== 256/512/1024)


def _allowed(diff):
    return (diff == 0) | ((diff > 0) & ((diff & (diff - 1)) == 0))


def _n_far(qb):
    return sum(1 for d in FAR if qb - d >= 0)


def build_program(has_bias: bool, has_am: bool):
    nc = bacc.Bacc("TRN2", target_bir_lowering=False)


    x_d = nc.declare_dram_parameter("x", [S, H], BF16, isOutput=False)
    wq_d = nc.declare_dram_parameter("wq", [128, KCH, GD], BF16, isOutput=False)
    wk_d = nc.declare_dram_parameter("wk", [128, KCH, GD], BF16, isOutput=False)
    wv_d = nc.declare_dram_parameter("wv", [128, KCH, GD], BF16, isOutput=False)
    # dense ADDITIVE log-masks per key block, replicated x2 so one N=512
    # matmul (identity stationary) adds them to a whole psum bank:
    # [pj, kb, rep, 256]
    masks_d = nc.declare_dram_parameter("masks", [128, NQB, 2, 256], BF16, isOutput=False)
    eye_d = nc.declare_dram_parameter("eye", [128, 128], BF16, isOutput=False)
    if has_am:
        amt_d = nc.declare_dram_parameter("amt", [128, NQB], F32, isOutput=False)
    if has_bias:
        bqm_d = nc.declare_dram_parameter("bqm", [1, 4, 128], BF16, isOutput=False)
        bkm_d = nc.declare_dram_parameter("bkm", [1, 4, 128], BF16, isOutput=False)
        bv_d = nc.declare_dram_parameter("bv", [1, GD], BF16, isOutput=False)
        ones_row_d = nc.declare_dram_parameter(
            "ones_row", [1, 512], BF16, isOutput=False
        )
    out_d = nc.declare_dram_parameter("out", [S, GD], F32, isOutput=True)

    with TileContext(nc) as tc:
        with (
            tc.tile_pool(name="const", bufs=1) as const_pool,
            tc.tile_pool(name="big", bufs=1) as big_pool,
            tc.tile_pool(name="far_sb", bufs=3) as far_pool,
        ):
            # ---- resident SBUF tensors ----
            qt = big_pool.tile([128, 4, S], BF16, tag="qt")  # [dh%128, m, s]
            kt = big_pool.tile([128, 4, S], BF16, tag="kt")
            # s-major copies for far diagonals, one tile per dh-slab m so
            # every DMA transpose writes a FULL tile (write-footprint dep
            # tracking on sliced transpose outputs proved unreliable):
            # qs_t[m][p, blk, r] = Q[blk*128+p, m*128+r] (heads 2m, 2m+1)
            qs_t = [
                big_pool.tile([128, NQB, 128], BF16, tag=f"qs{m}", name=f"qs{m}")
                for m in range(4)
            ]
            ks_t = [
                big_pool.tile([128, NQB, 128], BF16, tag=f"ks{m}", name=f"ks{m}")
                for m in range(4)
            ]
            vv = big_pool.tile([128, NQB, HPC, HD + 1], BF16, tag="v")
            masks = const_pool.tile([128, NQB, 2, 256], BF16, tag="masks")
            eye = const_pool.tile([128, 128], BF16, tag="eye")
            # far scores / probs, slab-major [s%128, slab, far_idx, qb, j]
            # (reduce/exp outputs stay contiguous; multi-dim strided
            # tiny-run outputs mis-execute on ACT/DVE)
            pfar_s = big_pool.tile([128, 4, 3, NQB, 2], F32, tag="pfar_s")
            pfar = big_pool.tile([128, 4, 3, NQB, 2], BF16, tag="pfar")
            # qb-major copy for the MAC reads (gpsimd relayout)
            pfar2 = big_pool.tile([128, NQB, 3, HPC], BF16, tag="pfar2")

            # ---- loads ----
            # x transpose hogs the sync queue; weights/masks go on the
            # scalar engine's DMA queue so both stream concurrently.
            def _load_proj_inputs(xt, wq, wk, wv):
                nc.sync.dma_start_transpose(xt[:, 0:4, :], x_d[:, 0:512])
                nc.scalar.dma_start_transpose(xt[:, 4:8, :], x_d[:, 512:1024])
                nc.scalar.dma_start(wq[:], wq_d[:])
                nc.scalar.dma_start(wk[:], wk_d[:])
                nc.scalar.dma_start(wv[:], wv_d[:])

            nc.scalar.dma_start(masks[:], masks_d[:])
            nc.scalar.dma_start(eye[:], eye_d[:])
            nc.vector.memset(vv[:, :, :, HD : HD + 1], 1.0)
            if has_am:
                amt = const_pool.tile([128, NQB], F32, tag="amt")
                nc.scalar.dma_start(amt[:], amt_d[:])
            if has_bias:
                bqm = const_pool.tile([1, 4, 128], BF16, tag="bqm")
                bkm = const_pool.tile([1, 4, 128], BF16, tag="bkm")
                bvr = const_pool.tile([1, GD], BF16, tag="bvr")
                ones_row = const_pool.tile([1, 512], BF16, tag="ones_row")
                nc.scalar.dma_start(bqm[:], bqm_d[:])
                nc.scalar.dma_start(bkm[:], bkm_d[:])
                nc.scalar.dma_start(bvr[:], bv_d[:])
                nc.scalar.dma_start(ones_row[:], ones_row_d[:])

            def _far_scores(m):
                """Far-diagonal scores for dh-slab m (heads 2m, 2m+1):
                per offset d, ONE batched DVE product over all query
                blocks, one segmented reduce over dh, one exp."""
                for di, d in enumerate(FAR):
                    fprod = far_pool.tile(
                        [128, NQB - d, 2, HD], BF16, tag=f"fprod{d}", name=f"fp{m}_{d}"
                    )
                    nc.vector.tensor_mul(
                        fprod.rearrange("p b h d -> p b (h d)"),
                        qs_t[m][:, d:NQB],
                        ks_t[m][:, 0 : NQB - d],
                    )
                    nc.vector.tensor_reduce(
                        pfar_s[:, m, di, d:NQB, :],
                        fprod[:],
                        axis=mybir.AxisListType.X,
                        op=mybir.AluOpType.add,
                    )

            def _far_exp():
                """exp of far scores (contiguous slab-major slices), then
                one gpsimd software-walk relayout into qb-major pfar2 for
                the MAC broadcast reads."""
                for m in range(4):
                    for di, d in enumerate(FAR):
                        if has_am:
                            for qb in range(d, NQB):
                                nc.scalar.activation(
                                    pfar[:, m, di, qb, :],
                                    pfar_s[:, m, di, qb, :],
                                    mybir.ActivationFunctionType.Exp,
                                    scale=0.125,
                                    bias=amt[:, qb - d : qb - d + 1],
                                )
                        else:
                            nc.scalar.activation(
                                pfar[:, m, di, d:NQB, :],
                                pfar_s[:, m, di, d:NQB, :],
                                mybir.ActivationFunctionType.Exp,
                                scale=0.125,
                            )
                nc.gpsimd.tensor_copy(
                    pfar2.rearrange("p q d (m j) -> p q d m j", m=4),
                    pfar.rearrange("p m d q j -> p q d m j"),
                )

            # ---- projections: QT/KT [dh, s] ----
            # xt + weights live only for this section; closing the pool
            # frees ~56KB/partition for the attention pools below
            with (
                tc.tile_pool(name="proj_sb", bufs=1) as proj_pool,
                tc.tile_pool(name="ppsum", bufs=4, space="PSUM") as ppsum,
            ):
                xt = proj_pool.tile([128, KCH, S], BF16, tag="xt")
                wq = proj_pool.tile([128, KCH, GD], BF16, tag="wq")
                wk = proj_pool.tile([128, KCH, GD], BF16, tag="wk")
                wv = proj_pool.tile([128, KCH, GD], BF16, tag="wv")
                _load_proj_inputs(xt, wq, wk, wv)
                # PE warmup: dependency-free dummy matmuls that run during
                # the startup DMA wait so HAM reaches 8/8 clock before the
                # projections start.
                scratch = const_pool.tile([128, 512], BF16, tag="warm")
                nc.vector.memset(scratch[:], 0.0)
                for wi in range(32):
                    wps = ppsum.tile([128, 512], F32, tag="pp")
                    nc.tensor.matmul(
                        wps[:],
                        scratch[:, 0:128],
                        scratch[:],
                        start=True,
                        stop=True,
                        skip_group_check=True,
                    )
                for m in range(4):  # dh 128-row tiles (2 heads each)
                    for n in range(4):  # s 512-col chunks
                        for dst, w, bias in ((qt, wq, "q"), (kt, wk, "k")):
                            ps = ppsum.tile([128, 512], F32, tag="pp")
                            for c in range(KCH):
                                nc.tensor.matmul(
                                    ps[:],
                                    w[:, c, m * 128 : (m + 1) * 128],
                                    xt[:, c, n * 512 : (n + 1) * 512],
                                    start=(c == 0),
                                    stop=(c == KCH - 1 and not has_bias),
                                )
                            if has_bias:
                                brow = bqm if bias == "q" else bkm
                                nc.tensor.matmul(
                                    ps[:],
                                    brow[:, m, :],
                                    ones_row[:],
                                    start=False,
                                    stop=True,
                                )
                            nc.scalar.activation(
                                dst[:, m, n * 512 : (n + 1) * 512],
                                ps[:],
                                mybir.ActivationFunctionType.Copy,
                            )
                    # stream finished 128-row slabs to DRAM and read them
                    # back transposed (s-major) right away; q on the sync
                    # queue, k on the scalar queue so they overlap.
                    # direct SBUF->SBUF xbar transpose: both edges
                    # (ACT-write->DMA-read of qt, DMA-write->DVE-read of
                    # qs_t) are the reliably-tracked dependency classes
                    nc.scalar.dma_start_transpose(qs_t[m][:], qt[:, m, :])
                    nc.scalar.dma_start_transpose(ks_t[m][:], kt[:, m, :])
                    # far-diagonal scores for this slab's two heads —
                    # overlaps the remaining projections on the PE
                    _far_scores(m)
                # ---- V [s, dh] ----
                for t in range(NQB):
                    ps = ppsum.tile([128, 512], F32, tag="pp")
                    for c in range(KCH):
                        nc.tensor.matmul(
                            ps[:],
                            xt[:, c, t * 128 : (t + 1) * 128],
                            wv[:, c, :],
                            start=(c == 0),
                            stop=(c == KCH - 1 and not has_bias),
                        )
                    if has_bias:
                        nc.tensor.matmul(
                            ps[:], ones_row[:, :128], bvr[:], start=False, stop=True
                        )
                    nc.scalar.activation(
                        vv[:, t, :, 0:HD], ps[:], mybir.ActivationFunctionType.Copy
                    )
            _far_exp()

            # fence: full-tile DVE read of pfar -> every far exp (ACT) is
            # complete before any later DVE op (the far MACs) can issue
            fence = const_pool.tile([128, 1], F32, tag="fence")
            nc.vector.tensor_reduce(
                fence[:],
                pfar2[:],
                axis=mybir.AxisListType.XYZ,
                op=mybir.AluOpType.max,
            )

            # ---- dense attention (key-block major, heads batched) ----
            # sc tile = 1 psum bank, 2 heads; row-tiled matmul pairs
            # (h even K-rows 0:64, h odd 64:128) land in different banks.
            SLOTMAP = (0, 2, 1, 3)  # pair partners land in different banks

            def tidx(h):
                return h // 4

            def slot(h):
                return SLOTMAP[h % 4]

            with (
                tc.tile_pool(name="spsum", bufs=3, space="PSUM") as spsum,
                tc.tile_pool(name="opsum", bufs=1, space="PSUM") as opsum,
                tc.tile_pool(name="att_sb", bufs=4) as att_sb,
                tc.tile_pool(name="fin_sb", bufs=6) as fin_sb,
            ):
                strips = {}
                def _pv_finalize(qb):

                    pv = opsum.tile([128, 2, 512], F32, tag="pv")
                    for h in range(HPC):
                        half, idx = h // 4, h % 4
                        nc.tensor.matmul(
                            pv[:, half, idx * 65 : idx * 65 + 65],
                            strips[qb][:, tidx(h), slot(h), 0:128],
                            vv[:, qb, h, :],
                            start=True,
                            stop=(qb == 0),
                            skip_group_check=True,
                        )
                        if qb >= 1:
                            nc.tensor.matmul(
                                pv[:, half, idx * 65 : idx * 65 + 65],
                                strips[qb - 1][:, tidx(h), slot(h), 128:256],
                                vv[:, qb - 1, h, :],
                                start=False,
                                stop=True,
                                skip_group_check=True,
                            )
                    pv_v = pv[:, :, 0:260].rearrange("p a (i c) -> p a i c", i=4)
                    posb = fin_sb.tile([128, HPC, HD + 1], F32, tag="posb")
                    posb_v = posb.rearrange("p (a i) c -> p a i c", a=2)
                    nf = _n_far(qb)
                    if nf:
                        # far MACs (bf16, 2x DVE mode):
                        # facc[si, h, :] = sum_d p_d[si,h] * v[qb-d][si,h,:]
                        facc = fin_sb.tile([128, HPC, HD + 1], BF16, tag="facc")
                        nc.vector.tensor_mul(
                            facc[:],
                            vv[:, qb - FAR[0]],
                            pfar2[:, qb, 0, :, None].broadcast_to([128, HPC, HD + 1]),
                        )
                        for di, d in enumerate(FAR[:nf]):
                            if di == 0:
                                continue
                            mtmp = fin_sb.tile([128, HPC, HD + 1], BF16, tag="mtmp")
                            nc.vector.tensor_mul(
                                mtmp[:],
                                vv[:, qb - d],
                                pfar2[:, qb, di, :, None].broadcast_to(
                                    [128, HPC, HD + 1]
                                ),
                            )
                            nc.vector.tensor_add(facc[:], facc[:], mtmp[:])
                        nc.vector.tensor_add(
                            posb_v, pv_v, facc.rearrange("p (a i) c -> p a i c", a=2)
                        )
                    else:
                        nc.vector.tensor_copy(posb_v, pv_v)
                    rinv = fin_sb.tile([128, HPC, 1], F32, tag="rinv")
                    nc.vector.reciprocal(rinv[:], posb[:, :, HD : HD + 1])
                    outs_t = fin_sb.tile([128, HPC, HD], F32, tag="outs")
                    nc.vector.tensor_mul(
                        outs_t[:],
                        posb[:, :, 0:HD],
                        rinv[:].broadcast_to([128, HPC, HD]),
                    )
                    nc.sync.dma_start(
                        out_d[qb * 128 : (qb + 1) * 128, :],
                        outs_t.rearrange("p h c -> p (h c)"),
                    )

                for kb in range(NQB):
                    nd = 256 if kb + 1 < NQB else 128
                    scs = [
                        spsum.tile([128, 4, 256], F32, tag="sc", name=f"sc{kb}_{i}")
                        for i in range(2)
                    ]
                    pt = att_sb.tile([128, 2, 4, 256], BF16, tag="pt")
                    for h in range(HPC):
                        mh, p0 = h // 2, (h % 2) * 64
                        nc.tensor.matmul(
                            scs[tidx(h)][:, slot(h), 0:nd],
                            kt[p0 : p0 + 64, mh, kb * 128 : (kb + 1) * 128],
                            qt[p0 : p0 + 64, mh, kb * 128 : kb * 128 + nd],
                            # start only for the first matmul touching each
                            # psum bank: start=True clears has_written for
                            # the WHOLE bank, which would make the bank-wide
                            # mask-add overwrite the other slot's scores
                            start=(h % 4 < 2),
                            stop=False,
                            skip_group_check=True,
                        )
                    # additive logsparse mask via identity-stationary matmul
                    # (frees the vector engine of the mask multiply)
                    for t in range(2):
                        for bank in range(2):
                            nc.tensor.matmul(
                                scs[t][:, 2 * bank : 2 * bank + 2, 0:nd],
                                eye[:],
                                masks[:, kb, :, 0:nd],
                                start=False,
                                stop=True,
                                skip_group_check=True,
                            )
                    for t in range(2):
                        nc.scalar.activation(
                            pt[:, t, :, 0:nd],
                            scs[t][:, :, 0:nd],
                            mybir.ActivationFunctionType.Exp,
                            scale=0.125,
                        )
                    strips[kb] = pt
                    if kb >= 1:
                        _pv_finalize(kb - 1)
                _pv_finalize(NQB - 1)
    nc.compile()
    return nc


_CACHE = {}


def _get_program(has_bias, has_am):
    key = (has_bias, has_am)
    if key not in _CACHE:
        _CACHE[key] = build_program(has_bias, has_am)
    return _CACHE[key]


def _host_masks(attention_mask_b):
    """Dense ADDITIVE log-mask strips [128, NQB, 2, 256] (f32), added to
    the score psum pre-exp: 0 where allowed else -1e9, plus 8*amask[j]
    (per key j = partition) so exp(0.125*(s+M)) = exp(0.125*s)*exp(am)."""
    pi = np.arange(128)[None, :]
    pj = np.arange(128)[:, None]
    pat = {}
    for dlt in (0, 1):
        pat[dlt] = np.where(
            _allowed(dlt * 128 + pi - pj), 0.0, -1e9
        ).astype(np.float32)
    am8 = 8.0 * attention_mask_b.astype(np.float32)  # [S]
    m = np.full((128, NQB, 256), -1e9, dtype=np.float32)
    for kb in range(NQB):
        amw = am8[kb * 128 : (kb + 1) * 128][:, None]  # [pj, 1]
        m[:, kb, 0:128] = pat[0] + amw
        if kb + 1 < NQB:
            m[:, kb, 128:256] = pat[1] + amw
    return np.repeat(m[:, :, None, :], 2, axis=2)


def _build_in_maps(
    hidden_states, attention_mask, Wq, bq, Wk, bk, Wv, bv, has_bias, has_am
):
    in_maps = []
    for c in range(8):
        b, g = c // 2, c % 2
        sl = slice(g * GD, (g + 1) * GD)
        im = {
            "x": hidden_states[b].astype(NPBF16),
            "wq": np.ascontiguousarray(
                Wq[sl, :].T.reshape(KCH, 128, GD).transpose(1, 0, 2)
            ).astype(NPBF16),
            "wk": np.ascontiguousarray(
                Wk[sl, :].T.reshape(KCH, 128, GD).transpose(1, 0, 2)
            ).astype(NPBF16),
            "wv": np.ascontiguousarray(
                Wv[sl, :].T.reshape(KCH, 128, GD).transpose(1, 0, 2)
            ).astype(NPBF16),
            "masks": _host_masks(attention_mask[b, 0, 0, :]).astype(NPBF16),
            "eye": np.eye(128, dtype=NPBF16),
        }
        if has_am:
            im["amt"] = np.ascontiguousarray(
                attention_mask[b, 0, 0, :].astype(np.float32).reshape(NQB, 128).T
            )
        if has_bias:
            im["bqm"] = bq[sl].reshape(1, 4, 128).astype(NPBF16)
            im["bkm"] = bk[sl].reshape(1, 4, 128).astype(NPBF16)
            im["bv"] = bv[sl].reshape(1, GD).astype(NPBF16)
            im["ones_row"] = np.ones((1, 512), dtype=NPBF16)
        in_maps.append(im)
    return in_maps


def kernel(hidden_states, attention_mask, Wq, bq, Wk, bk, Wv, bv, _trace=False):
    hidden_states = np.asarray(hidden_states)
    attention_mask = np.asarray(attention_mask)
    Wq, bq = np.asarray(Wq), np.asarray(bq)
    Wk, bk = np.asarray(Wk), np.asarray(bk)
    Wv, bv = np.asarray(Wv), np.asarray(bv)

    has_bias = bool(np.any(bq) or np.any(bk) or np.any(bv))
    has_am = bool(np.any(attention_mask))
    nc = _get_program(has_bias, has_am)
    in_maps = _build_in_maps(
        hidden_states, attention_mask, Wq, bq, Wk, bk, Wv, bv, has_bias, has_am
    )

    kw = {}
    if _trace:
        import os
        import shutil

        shutil.rmtree("/tmp/bass_trace", ignore_errors=True)
        os.makedirs("/tmp/bass_trace", exist_ok=True)
        kw = dict(tmpdir="/tmp/bass_trace")
    res = run_bass_kernel_spmd(nc, in_maps, list(range(8)), trace=_trace, **kw)
    out = np.empty((B, S, H), dtype=np.float32)
    for c in range(8):
        b, g = c // 2, c % 2
        out[b, :, g * GD : (g + 1) * GD] = res.results[c]["out"]
    if _trace:
        return out, res
    return out


# revision 64
# speedup vs baseline: 1.0137x; 1.0137x over previous
"""LogSparse attention kernel for 8 TRN2 NeuronCores.

Problem: B=4, S=2048, H=1024, 16 heads x 64 dim. Logsparse mask: query i
attends key j iff i-j == 0 or i-j == 2^k (so <=12 keys per query, at
power-of-2 offsets).

Sharding: core c -> batch b = c//2, head-group g = c%2 (8 heads each).
Each core computes q/k/v projections for its (batch, head-group) and the
sparse attention, writing out[b, :, g*512:(g+1)*512].

Device algorithm (per core):
  - DMA-transpose X (bf16, two halves on the two DMA queues) -> XT [h, s].
  - QT/KT = W @ XT ([dh, s], dh on partitions), V = X @ WvT (s-major,
    with a ones column for row sums). After each 128-row slab of QT/KT,
    an SBUF->SBUF xbar transpose produces s-major per-slab copies
    qs_t/ks_t [s%128, blk, 128] (whole-tile transposes only: sliced
    transpose outputs and DRAM round-trips have unreliable DMA ordering).
  - Far diagonals (offsets 256/512/1024) only need diag(Q Kshift^T):
    batched DVE products of s-major q/k over all query blocks + one
    segmented tensor_reduce per (slab, offset) -> scores [si, qb, d, 2],
    exp'd on ACT, relayed out to qb-major via one gpsimd copy. All of it
    runs during the remaining projections on otherwise-idle engines.
    (Reduce/exp slices stay contiguous: multi-dim strided tiny-run
    outputs mis-execute on ACT/DVE.)
  - Dense attention is key-block-major: key block kb serves query blocks
    kb and kb+1 (256 score columns). Per kb: 8 score matmuls (K=64,
    row-tiled pairs into different psum banks, start= only on the first
    matmul per bank), then the logsparse mask is ADDED on the PE itself
    (identity-stationary matmuls adding -1e9/8*am log-masks into the
    psum) so exp(0.125*(s+M)) masks exactly to 0 with no vector-engine
    work; two batched exp ACTs (psum->bf16) per kb.
  - PV: per qb, 16 matmuls (2 strips x 8 heads, N=65 incl rowsum col)
    accumulate into 2 psum banks; far p*v rank-1 MACs (broadcast APs),
    psum+far combine, reciprocal and normalize are batched per qb on DVE.
Softmax max-subtraction is skipped: scores*0.125 has std ~0.4 for this
problem family, far from exp overflow.
"""

import numpy as np
import ml_dtypes

import concourse.bass as bass
from concourse import bacc
import concourse.mybir as mybir
from concourse.tile import TileContext
from concourse.bass_utils import run_bass_kernel_spmd

B, S, H = 4, 2048, 1024
NH, HD = 16, 64
G = 2  # head groups per batch
HPC = NH // G  # heads per core = 8
GD = HPC * HD  # 512 group dim
NQB = S // 128  # 16 query blocks
KCH = H // 128  # 8 contraction chunks

BF16 = mybir.dt.bfloat16
F32 = mybir.dt.float32
NPBF16 = ml_dtypes.bfloat16

FAR = (2, 4, 8)  # far diagonal offsets in 128-blocks (== 256/512/1024)


def _allowed(diff):
    return (diff == 0) | ((diff > 0) & ((diff & (diff - 1)) == 0))


def _n_far(qb):
    return sum(1 for d in FAR if qb - d >= 0)


def build_program(has_bias: bool, has_am: bool):
    nc = bacc.Bacc("TRN2", target_bir_lowering=False)


    x_d = nc.declare_dram_parameter("x", [S, H], BF16, isOutput=False)
    wq_d = nc.declare_dram_parameter("wq", [128, KCH, GD], BF16, isOutput=False)
    wk_d = nc.declare_dram_parameter("wk", [128, KCH, GD], BF16, isOutput=False)
    wv_d = nc.declare_dram_parameter("wv", [128, KCH, GD], BF16, isOutput=False)
    # dense ADDITIVE log-masks per key block, replicated x2 so one N=512
    # matmul (identity stationary) adds them to a whole psum bank:
    # [pj, kb, rep, 256]
    masks_d = nc.declare_dram_parameter("masks", [128, NQB, 2, 256], BF16, isOutput=False)
    eye_d = nc.declare_dram_parameter("eye", [128, 128], BF16, isOutput=False)
    if has_am:
        amt_d = nc.declare_dram_parameter("amt", [128, NQB], F32, isOutput=False)
    if has_bias:
        bqm_d = nc.declare_dram_parameter("bqm", [1, 4, 128], BF16, isOutput=False)
        bkm_d = nc.declare_dram_parameter("bkm", [1, 4, 128], BF16, isOutput=False)
        bv_d = nc.declare_dram_parameter("bv", [1, GD], BF16, isOutput=False)
        ones_row_d = nc.declare_dram_parameter(
            "ones_row", [1, 512], BF16, isOutput=False
        )
    out_d = nc.declare_dram_parameter("out", [S, GD], F32, isOutput=True)

    with TileContext(nc) as tc:
        with (
            tc.tile_pool(name="const", bufs=1) as const_pool,
            tc.tile_pool(name="big", bufs=1) as big_pool,
            tc.tile_pool(name="far_sb", bufs=3) as far_pool,
        ):
            # ---- resident SBUF tensors ----
            qt = big_pool.tile([128, 4, S], BF16, tag="qt")  # [dh%128, m, s]
            kt = big_pool.tile([128, 4, S], BF16, tag="kt")
            # s-major copies for far diagonals, one tile per dh-slab m so
            # every DMA transpose writes a FULL tile (write-footprint dep
            # tracking on sliced transpose outputs proved unreliable):
            # qs_t[m][p, blk, r] = Q[blk*128+p, m*128+r] (heads 2m, 2m+1)
            qs_t = [
                big_pool.tile([128, NQB, 128], BF16, tag=f"qs{m}", name=f"qs{m}")
                for m in range(4)
            ]
            ks_t = [
                big_pool.tile([128, NQB, 128], BF16, tag=f"ks{m}", name=f"ks{m}")
                for m in range(4)
            ]
            vv = big_pool.tile([128, NQB, HPC, HD + 1], BF16, tag="v")
            masks = const_pool.tile([128, NQB, 2, 256], BF16, tag="masks")
            eye = const_pool.tile([128, 128], BF16, tag="eye")
            # far scores / probs, slab-major [s%128, slab, far_idx, qb, j]
            # (reduce/exp outputs stay contiguous; multi-dim strided
            # tiny-run outputs mis-execute on ACT/DVE)
            pfar_s = big_pool.tile([128, 4, 3, NQB, 2], F32, tag="pfar_s")
            pfar = big_pool.tile([128, 4, 3, NQB, 2], BF16, tag="pfar")
            # qb-major copy for the MAC reads (gpsimd relayout)
            pfar2 = big_pool.tile([128, NQB, 3, HPC], BF16, tag="pfar2")

            # ---- loads ----
            # x transpose hogs the sync queue; weights/masks go on the
            # scalar engine's DMA queue so both stream concurrently.
            def _load_proj_inputs(xt, wq, wk, wv):
                nc.sync.dma_start_transpose(xt[:, 0:4, :], x_d[:, 0:512])
                nc.scalar.dma_start_transpose(xt[:, 4:8, :], x_d[:, 512:1024])
                nc.scalar.dma_start(wq[:], wq_d[:])
                nc.scalar.dma_start(wk[:], wk_d[:])
                nc.scalar.dma_start(wv[:], wv_d[:])

            nc.scalar.dma_start(masks[:], masks_d[:])
            nc.scalar.dma_start(eye[:], eye_d[:])
            nc.vector.memset(vv[:, :, :, HD : HD + 1], 1.0)
            if has_am:
                amt = const_pool.tile([128, NQB], F32, tag="amt")
                nc.scalar.dma_start(amt[:], amt_d[:])
            if has_bias:
                bqm = const_pool.tile([1, 4, 128], BF16, tag="bqm")
                bkm = const_pool.tile([1, 4, 128], BF16, tag="bkm")
                bvr = const_pool.tile([1, GD], BF16, tag="bvr")
                ones_row = const_pool.tile([1, 512], BF16, tag="ones_row")
                nc.scalar.dma_start(bqm[:], bqm_d[:])
                nc.scalar.dma_start(bkm[:], bkm_d[:])
                nc.scalar.dma_start(bvr[:], bv_d[:])
                nc.scalar.dma_start(ones_row[:], ones_row_d[:])

            def _far_scores(m):
                """Far-diagonal scores for dh-slab m (heads 2m, 2m+1):
                per offset d, ONE batched DVE product over all query
                blocks, one segmented reduce over dh, one exp."""
                for di, d in enumerate(FAR):
                    fprod = far_pool.tile(
                        [128, NQB - d, 2, HD], BF16, tag=f"fprod{d}", name=f"fp{m}_{d}"
                    )
                    nc.vector.tensor_mul(
                        fprod.rearrange("p b h d -> p b (h d)"),
                        qs_t[m][:, d:NQB],
                        ks_t[m][:, 0 : NQB - d],
                    )
                    nc.vector.tensor_reduce(
                        pfar_s[:, m, di, d:NQB, :],
                        fprod[:],
                        axis=mybir.AxisListType.X,
                        op=mybir.AluOpType.add,
                    )

            def _far_exp():
                """exp of far scores (contiguous slab-major slices), then
                one gpsimd software-walk relayout into qb-major pfar2 for
                the MAC broadcast reads."""
                for m in range(4):
                    for di, d in enumerate(FAR):
                        if has_am:
                            for qb in range(d, NQB):
                                nc.scalar.activation(
                                    pfar[:, m, di, qb, :],
                                    pfar_s[:, m, di, qb, :],
                                    mybir.ActivationFunctionType.Exp,
                                    scale=0.125,
                                    bias=amt[:, qb - d : qb - d + 1],
                                )
                        else:
                            nc.scalar.activation(
                                pfar[:, m, di, d:NQB, :],
                                pfar_s[:, m, di, d:NQB, :],
                                mybir.ActivationFunctionType.Exp,
                                scale=0.125,
                            )
                nc.gpsimd.tensor_copy(
                    pfar2.rearrange("p q d (m j) -> p q d m j", m=4),
                    pfar.rearrange("p m d q j -> p q d m j"),
                )

            # ---- projections: QT/KT [dh, s] ----
            # xt + weights live only for this section; closing the pool
            # frees ~56KB/partition for the attention pools below
            with (
                tc.tile_pool(name="proj_sb", bufs=1) as proj_pool,
                tc.tile_pool(name="ppsum", bufs=4, space="PSUM") as ppsum,
            ):
                xt = proj_pool.tile([128, KCH, S], BF16, tag="xt")
                wq = proj_pool.tile([128, KCH, GD], BF16, tag="wq")
                wk = proj_pool.tile([128, KCH, GD], BF16, tag="wk")
                wv = proj_pool.tile([128, KCH, GD], BF16, tag="wv")
                _load_proj_inputs(xt, wq, wk, wv)
                # PE warmup: dependency-free dummy matmuls that run during
                # the startup DMA wait so HAM reaches 8/8 clock before the
                # projections start.
                scratch = const_pool.tile([128, 512], BF16, tag="warm")
                nc.vector.memset(scratch[:], 0.0)
                for wi in range(32):
                    wps = ppsum.tile([128, 512], F32, tag="pp")
                    nc.tensor.matmul(
                        wps[:],
                        scratch[:, 0:128],
                        scratch[:],
                        start=True,
                        stop=True,
                        skip_group_check=True,
                    )
                for m in range(4):  # dh 128-row tiles (2 heads each)
                    for n in range(4):  # s 512-col chunks
                        for dst, w, bias in ((qt, wq, "q"), (kt, wk, "k")):
                            ps = ppsum.tile([128, 512], F32, tag="pp")
                            for c in range(KCH):
                                nc.tensor.matmul(
                                    ps[:],
                                    w[:, c, m * 128 : (m + 1) * 128],
                                    xt[:, c, n * 512 : (n + 1) * 512],
                                    start=(c == 0),
                                    stop=(c == KCH - 1 and not has_bias),
                                )
                            if has_bias:
                                brow = bqm if bias == "q" else bkm
                                nc.tensor.matmul(
                                    ps[:],
                                    brow[:, m, :],
                                    ones_row[:],
                                    start=False,
                                    stop=True,
                                )
                            nc.scalar.activation(
                                dst[:, m, n * 512 : (n + 1) * 512],
                                ps[:],
                                mybir.ActivationFunctionType.Copy,
                            )
                    # stream finished 128-row slabs to DRAM and read them
                    # back transposed (s-major) right away; q on the sync
                    # queue, k on the scalar queue so they overlap.
                    # direct SBUF->SBUF xbar transpose: both edges
                    # (ACT-write->DMA-read of qt, DMA-write->DVE-read of
                    # qs_t) are the reliably-tracked dependency classes
                    nc.scalar.dma_start_transpose(qs_t[m][:], qt[:, m, :])
                    nc.scalar.dma_start_transpose(ks_t[m][:], kt[:, m, :])
                    # far-diagonal scores for this slab's two heads —
                    # overlaps the remaining projections on the PE
                    _far_scores(m)
                # ---- V [s, dh] ----
                for t in range(NQB):
                    ps = ppsum.tile([128, 512], F32, tag="pp")
                    for c in range(KCH):
                        nc.tensor.matmul(
                            ps[:],
                            xt[:, c, t * 128 : (t + 1) * 128],
                            wv[:, c, :],
                            start=(c == 0),
                            stop=(c == KCH - 1 and not has_bias),
                        )
                    if has_bias:
                        nc.tensor.matmul(
                            ps[:], ones_row[:, :128], bvr[:], start=False, stop=True
                        )
                    nc.scalar.activation(
                        vv[:, t, :, 0:HD], ps[:], mybir.ActivationFunctionType.Copy
                    )
            _far_exp()

            # fence: full-tile DVE read of pfar -> every far exp (ACT) is
            # complete before any later DVE op (the far MACs) can issue
            fence = const_pool.tile([128, 1], F32, tag="fence")
            nc.vector.tensor_reduce(
                fence[:],
                pfar2[:],
                axis=mybir.AxisListType.XYZ,
                op=mybir.AluOpType.max,
            )

            # ---- dense attention (key-block major, heads batched) ----
            # sc tile = 1 psum bank, 2 heads; row-tiled matmul pairs
            # (h even K-rows 0:64, h odd 64:128) land in different banks.
            SLOTMAP = (0, 2, 1, 3)  # pair partners land in different banks

            def tidx(h):
                return h // 4

            def slot(h):
                return SLOTMAP[h % 4]

            with (
                tc.tile_pool(name="spsum", bufs=2, space="PSUM") as spsum,
                tc.tile_pool(name="opsum", bufs=2, space="PSUM") as opsum,
                tc.tile_pool(name="att_sb", bufs=4) as att_sb,
                tc.tile_pool(name="fin_sb", bufs=6) as fin_sb,
            ):
                strips = {}
                def _pv_finalize(qb):

                    pv = opsum.tile([128, 2, 512], F32, tag="pv")
                    for h in range(HPC):
                        half, idx = h // 4, h % 4
                        nc.tensor.matmul(
                            pv[:, half, idx * 65 : idx * 65 + 65],
                            strips[qb][:, tidx(h), slot(h), 0:128],
                            vv[:, qb, h, :],
                            start=True,
                            stop=(qb == 0),
                            skip_group_check=True,
                        )
                        if qb >= 1:
                            nc.tensor.matmul(
                                pv[:, half, idx * 65 : idx * 65 + 65],
                                strips[qb - 1][:, tidx(h), slot(h), 128:256],
                                vv[:, qb - 1, h, :],
                                start=False,
                                stop=True,
                                skip_group_check=True,
                            )
                    pv_v = pv[:, :, 0:260].rearrange("p a (i c) -> p a i c", i=4)
                    posb = fin_sb.tile([128, HPC, HD + 1], F32, tag="posb")
                    posb_v = posb.rearrange("p (a i) c -> p a i c", a=2)
                    nf = _n_far(qb)
                    if nf:
                        # far MACs (bf16, 2x DVE mode):
                        # facc[si, h, :] = sum_d p_d[si,h] * v[qb-d][si,h,:]
                        facc = fin_sb.tile([128, HPC, HD + 1], BF16, tag="facc")
                        nc.vector.tensor_mul(
                            facc[:],
                            vv[:, qb - FAR[0]],
                            pfar2[:, qb, 0, :, None].broadcast_to([128, HPC, HD + 1]),
                        )
                        for di, d in enumerate(FAR[:nf]):
                            if di == 0:
                                continue
                            mtmp = fin_sb.tile([128, HPC, HD + 1], BF16, tag="mtmp")
                            nc.vector.tensor_mul(
                                mtmp[:],
                                vv[:, qb - d],
                                pfar2[:, qb, di, :, None].broadcast_to(
                                    [128, HPC, HD + 1]
                                ),
                            )
                            nc.vector.tensor_add(facc[:], facc[:], mtmp[:])
                        nc.vector.tensor_add(
                            posb_v, pv_v, facc.rearrange("p (a i) c -> p a i c", a=2)
                        )
                    else:
                        nc.vector.tensor_copy(posb_v, pv_v)
                    rinv = fin_sb.tile([128, HPC, 1], F32, tag="rinv")
                    nc.vector.reciprocal(rinv[:], posb[:, :, HD : HD + 1])
                    outs_t = fin_sb.tile([128, HPC, HD], F32, tag="outs")
                    nc.vector.tensor_mul(
                        outs_t[:],
                        posb[:, :, 0:HD],
                        rinv[:].broadcast_to([128, HPC, HD]),
                    )
                    nc.sync.dma_start(
                        out_d[qb * 128 : (qb + 1) * 128, :],
                        outs_t.rearrange("p h c -> p (h c)"),
                    )

                for kb in range(NQB):
                    nd = 256 if kb + 1 < NQB else 128
                    scs = [
                        spsum.tile([128, 4, 256], F32, tag="sc", name=f"sc{kb}_{i}")
                        for i in range(2)
                    ]
                    pt = att_sb.tile([128, 2, 4, 256], BF16, tag="pt")
                    for h in range(HPC):
                        mh, p0 = h // 2, (h % 2) * 64
                        nc.tensor.matmul(
                            scs[tidx(h)][:, slot(h), 0:nd],
                            kt[p0 : p0 + 64, mh, kb * 128 : (kb + 1) * 128],
                            qt[p0 : p0 + 64, mh, kb * 128 : kb * 128 + nd],
                            # start only for the first matmul touching each
                            # psum bank: start=True clears has_written for
                            # the WHOLE bank, which would make the bank-wide
                            # mask-add overwrite the other slot's scores
                            start=(h % 4 < 2),
                            stop=False,
                            skip_group_check=True,
                        )
                    # additive logsparse mask via identity-stationary matmul
                    # (frees the vector engine of the mask multiply)
                    for t in range(2):
                        for bank in range(2):
                            nc.tensor.matmul(
                                scs[t][:, 2 * bank : 2 * bank + 2, 0:nd],
                                eye[:],
                                masks[:, kb, :, 0:nd],
                                start=False,
                                stop=True,
                                skip_group_check=True,
                            )
                    for t in range(2):
                        nc.scalar.activation(
                            pt[:, t, :, 0:nd],
                            scs[t][:, :, 0:nd],
                            mybir.ActivationFunctionType.Exp,
                            scale=0.125,
                        )
                    strips[kb] = pt
                    if kb >= 1:
                        _pv_finalize(kb - 1)
                _pv_finalize(NQB - 1)
    nc.compile()
    return nc


_CACHE = {}


def _get_program(has_bias, has_am):
    key = (has_bias, has_am)
    if key not in _CACHE:
        _CACHE[key] = build_program(has_bias, has_am)
    return _CACHE[key]


def _host_masks(attention_mask_b):
    """Dense ADDITIVE log-mask strips [128, NQB, 2, 256] (f32), added to
    the score psum pre-exp: 0 where allowed else -1e9, plus 8*amask[j]
    (per key j = partition) so exp(0.125*(s+M)) = exp(0.125*s)*exp(am)."""
    pi = np.arange(128)[None, :]
    pj = np.arange(128)[:, None]
    pat = {}
    for dlt in (0, 1):
        pat[dlt] = np.where(
            _allowed(dlt * 128 + pi - pj), 0.0, -1e9
        ).astype(np.float32)
    am8 = 8.0 * attention_mask_b.astype(np.float32)  # [S]
    m = np.full((128, NQB, 256), -1e9, dtype=np.float32)
    for kb in range(NQB):
        amw = am8[kb * 128 : (kb + 1) * 128][:, None]  # [pj, 1]
        m[:, kb, 0:128] = pat[0] + amw
        if kb + 1 < NQB:
            m[:, kb, 128:256] = pat[1] + amw
    return np.repeat(m[:, :, None, :], 2, axis=2)


def _build_in_maps(
    hidden_states, attention_mask, Wq, bq, Wk, bk, Wv, bv, has_bias, has_am
):
    in_maps = []
    for c in range(8):
        b, g = c // 2, c % 2
        sl = slice(g * GD, (g + 1) * GD)
        im = {
            "x": hidden_states[b].astype(NPBF16),
            "wq": np.ascontiguousarray(
                Wq[sl, :].T.reshape(KCH, 128, GD).transpose(1, 0, 2)
            ).astype(NPBF16),
            "wk": np.ascontiguousarray(
                Wk[sl, :].T.reshape(KCH, 128, GD).transpose(1, 0, 2)
            ).astype(NPBF16),
            "wv": np.ascontiguousarray(
                Wv[sl, :].T.reshape(KCH, 128, GD).transpose(1, 0, 2)
            ).astype(NPBF16),
            "masks": _host_masks(attention_mask[b, 0, 0, :]).astype(NPBF16),
            "eye": np.eye(128, dtype=NPBF16),
        }
        if has_am:
            im["amt"] = np.ascontiguousarray(
                attention_mask[b, 0, 0, :].astype(np.float32).reshape(NQB, 128).T
            )
        if has_bias:
            im["bqm"] = bq[sl].reshape(1, 4, 128).astype(NPBF16)
            im["bkm"] = bk[sl].reshape(1, 4, 128).astype(NPBF16)
            im["bv"] = bv[sl].reshape(1, GD).astype(NPBF16)
            im["ones_row"] = np.ones((1, 512), dtype=NPBF16)
        in_maps.append(im)
    return in_maps


def kernel(hidden_states, attention_mask, Wq, bq, Wk, bk, Wv, bv, _trace=False):
    hidden_states = np.asarray(hidden_states)
    attention_mask = np.asarray(attention_mask)
    Wq, bq = np.asarray(Wq), np.asarray(bq)
    Wk, bk = np.asarray(Wk), np.asarray(bk)
    Wv, bv = np.asarray(Wv), np.asarray(bv)

    has_bias = bool(np.any(bq) or np.any(bk) or np.any(bv))
    has_am = bool(np.any(attention_mask))
    nc = _get_program(has_bias, has_am)
    in_maps = _build_in_maps(
        hidden_states, attention_mask, Wq, bq, Wk, bk, Wv, bv, has_bias, has_am
    )

    kw = {}
    if _trace:
        import os
        import shutil

        shutil.rmtree("/tmp/bass_trace", ignore_errors=True)
        os.makedirs("/tmp/bass_trace", exist_ok=True)
        kw = dict(tmpdir="/tmp/bass_trace")
    res = run_bass_kernel_spmd(nc, in_maps, list(range(8)), trace=_trace, **kw)
    out = np.empty((B, S, H), dtype=np.float32)
    for c in range(8):
        b, g = c // 2, c % 2
        out[b, :, g * GD : (g + 1) * GD] = res.results[c]["out"]
    if _trace:
        return out, res
    return out
